# revision 92
# baseline (speedup 1.0000x reference)
"""Trainium2 Bass kernel for nn_MultiHeadAttention_4999341933079.

Multi-head attention, B=8, N=1024, dim=768, 16 heads, head_dim=48, with the
torch-faithful raw-memory reshapes:
    qkv  = x @ Wqkv                      # [B, N, 2304]
    q,k,v = raw_view(qkv, (3, B, 16, N, 48))
    out  = softmax(q k^T / sqrt(48)) v   -> raw_view -> @ Wo + bo

Sharding: data parallel over the OUTPUT batch.  Core j owns output batch j.
Because the raw reshape scrambles batches, core j's Q/K/V head slabs are three
contiguous flat ranges of the qkv GEMM output.  Each range covers ~342 rows of
the [8192, 2304] GEMM, so core j only computes 3x384 = 1152 GEMM rows (1/8 of
the work, zero collectives, zero redundancy up to padding).

Per-core pipeline (one SPMD program on 8 cores):
  Ph1: qkv GEMM (bf16) -> scatter into DRAM scratch at a dynamic offset
       (core-dependent sub-row shift r_s = (2s + j) % 3, from partition_id):
         zq/zk: [rows, 128] bf16, Q^T/K^T source data duplicated at cols 0-47
                and 64-111 so one DMA-transpose load yields both row-group
                copies for concurrently row-tiled matmuls,
         zv:    [rows, 48] bf16 natural layout.
  Ph2: per head h: DMA-transpose loads Q^T,K^T [128,1024]; S^T = K^T.T @ Q^T
       row-tiled over two 48-row groups; exp via ScalarE (scale folded in, no
       max subtraction -- |S| <= ~6), the write AP permuting the query axis
       into output-token order; C^T = [V|ones].T @ expS^T col-tiled 2 heads
       per pass, the ones column giving softmax denominators for free.
  Per 4 heads (software-pipelined against later heads so PE never waits on
  the recip chain): recip = exp(-ln(sums)) on ScalarE; broadcast via a 0/1
  selection matmul (f32r); normalize C^T; scatter to concat^T (DRAM); and,
  one group later, the per-pair output GEMM  out = concat^T.T @ Wo + bo
  (output tokens 128u..128u+127 depend only on head pair u).

Accepts FULL inputs, returns FULL output.  Host work is only
slice/transpose/cast packing (no FLOPs).
"""

import numpy as np
import ml_dtypes

import concourse.mybir as mybir
import concourse.tile as tile
from concourse import bacc
from concourse import bass_utils
from concourse.bass import ds

B = 8
SEQ = 1024
DIM = 768
HEADS = 16
DH = 48
SCALE = DH ** -0.5
C3 = 3 * DIM          # 2304
RROWS = 384           # packed x rows per range (>= 342 actually needed)
TROWS = 3 * RROWS     # 1152
ZBUF = RROWS * 48 + 32  # z-rows per range + shift headroom

BF16 = mybir.dt.bfloat16
F32 = mybir.dt.float32

# qkv GEMM free-dim chunks: 48-aligned (for the scatter copies) and <= 512
# (PSUM bank limit).  Chunks are packed two-per-PSUM-slot ([128, 2, 512] f32,
# chunk i at [:, i, :w] so each stays within one 2KB bank) so an m-tile needs
# 3 pool slots instead of 5 -- the 4-slot psum pool never stalls the PE.
CHUNK_GROUPS = [((0, 480), (480, 480)), ((960, 480), (1440, 480)),
                ((1920, 384),)]

TRACE = False
TRACE_ALL_CORES = True
LAST_EXEC_NS = None
LAST_RESULTS = None


def _kernel_body(nc, tc, xT, wq, wo, bo, sel, out, zq, zk, zv, concd):
    Exp = mybir.ActivationFunctionType.Exp
    F32R = mybir.dt.float32r

    # Core-dependent sub-row shift r_s = (2s + core) % 3 applied as a
    # dynamic DRAM offset on the z-buffer stores.  Each issuing engine
    # needs its own partition_id register (APs bind registers per engine).
    pid = nc.gpsimd.partition_id()
    shift_rows = [32 - 16 * ((2 * s + pid) % 3) for s in range(3)]
    pid_sp = nc.sync.partition_id()
    shift_rows_sp = [32 - 16 * ((2 * s + pid_sp) % 3) for s in range(3)]
    pid_sc = nc.scalar.partition_id()
    shift_rows_sc = [32 - 16 * ((2 * s + pid_sc) % 3) for s in range(3)]

    with tc.tile_pool(name="const", bufs=1) as constp, \
         tc.tile_pool(name="work", bufs=2) as work, \
         tc.tile_pool(name="pexp", bufs=2) as pexp, \
         tc.tile_pool(name="pct", bufs=4) as pct, \
         tc.tile_pool(name="psall", bufs=4, space="PSUM") as psall:
        # Loads are ordered so the phase-1 GEMM streams: xt narrow-first
        # (m-tile s=0 columns of every k land in ~1.6us) on gpsimd, the
        # first chunk-group of wq columns on the sync queue, the remaining
        # wq column groups on the scalar HWDGE queue, and the late-used
        # wo/bo/sel after all wq traffic.
        wo_sb = constp.tile([128, 6, DIM], BF16)
        bo_sb = constp.tile([128, DIM], F32)
        # Persistent normalize state (memset once, rewritten per head pair):
        # sums2 rows 32-48 / 96-112 receive the C^T ones-column copies (sums
        # at rows 48/112, zeros elsewhere); selS rows 48/112 broadcast the
        # sums to the two 48-row groups in one matmul per ci half.
        sums2 = constp.tile([128, 2, 512], F32R)
        sel2_sb = constp.tile([128, 128], F32R)
        xt_sb = constp.tile([128, 6, TROWS], BF16)
        wq_sb = constp.tile([128, 6, C3], BF16)
        xt_r = xT.rearrange("(q p) t -> p q t", p=128)
        wq_r = wq.rearrange("(q p) c -> p q c", p=128)
        GCOLS = [(0, 960), (960, 960), (1920, 384)]
        for k in range(6):
            nc.gpsimd.dma_start(xt_sb[:, k, 0:RROWS], xt_r[:, k, 0:RROWS])
            nc.sync.dma_start(wq_sb[:, k, 0:960], wq_r[:, k, 0:960])
        nc.vector.memset(sums2.bitcast(mybir.dt.uint32), 0)
        warmt = constp.tile([1, 8], F32)
        nc.vector.memset(warmt, 0.0)
        for g0, gw in GCOLS[1:]:
            for k in range(6):
                nc.scalar.dma_start(wq_sb[:, k, g0:g0 + gw],
                                    wq_r[:, k, g0:g0 + gw])
        for k in range(6):
            nc.gpsimd.dma_start(xt_sb[:, k, RROWS:TROWS],
                                xt_r[:, k, RROWS:TROWS])

        def late_loads():
            # Issued mid-schedule (after mg(2,0)) on the gpsimd queue so
            # these transfers never contend with the phase-1 wq/xt streams
            # or block the scalar-engine dup copies.
            nc.gpsimd.dma_start(wo_sb,
                                wo.rearrange("(q p) f -> p q f", p=128))
            nc.gpsimd.dma_start(bo_sb, bo)
            nc.gpsimd.dma_start(sel2_sb, sel)

        # ---------- Phase 1: qkv GEMM + scatter to z buffers ----------
        # Manually-rotated staging buffers; pad cols initialized once.
        # zq/zk data is duplicated at cols 0-47 and 64-111 (the DMA
        # transpose needs a 128-col source, and the two copies feed the
        # concurrently row-tiled S matmuls at partitions 0/64).
        stgs = [constp.tile([128, 48, 128], BF16, name=f"stg{i}")
                for i in range(2)]
        for t in stgs:
            nc.vector.memset(t[:, :, 48:64], 0.0)
            nc.vector.memset(t[:, :, 112:128], 0.0)

        M_state = {}
        _stg_ctr = [0]

        def emit_mg(s, ttile, gi):
            """One chunk-group (~1.9us of PE) of the (s,ttile) m-tile:
            matmuls, drain copies, and on the last group the z scatter."""
            t0 = s * RROWS + ttile * 128
            key = (s, ttile)
            if gi == 0:
                if s < 2:
                    stg = stgs[_stg_ctr[0] % 2]
                    _stg_ctr[0] += 1
                else:
                    stg = work.tile([128, 48, DH], BF16, tag="stgv",
                                    name=f"stgv_{ttile}")
                M_state[key] = stg
            stg = M_state[key]
            grp = CHUNK_GROUPS[gi]
            pg = psall.tile([128, 2, 512], F32, tag="ps",
                            name=f"psc_{s}_{ttile}_{gi}")
            for k in range(6):
                lhsT = xt_sb[:, k, t0:t0 + 128]
                for ci, (c0, cw) in enumerate(grp):
                    nc.tensor.matmul(
                        pg[:, ci, 0:cw], lhsT,
                        wq_sb[:, k, c0:c0 + cw],
                        start=(k == 0), stop=(k == 5))
            for ci, (c0, cw) in enumerate(grp):
                j0, nj = c0 // DH, cw // DH
                src_ap = pg[:, ci, 0:cw].rearrange("p (j d) -> p j d", d=DH)
                nc.vector.tensor_copy(
                    out=stg[:, j0:j0 + nj, 0:DH], in_=src_ap)
                if s < 2:
                    # duplicate row-group copy; ScalarE takes it only for
                    # the pre-blk m-tiles (its queue is exp-only later)
                    if (s, ttile) in ((0, 0), (1, 0), (0, 1)):
                        nc.scalar.copy(
                            out=stg[:, j0:j0 + nj, 64:64 + DH], in_=src_ap)
                    else:
                        nc.vector.tensor_copy(
                            out=stg[:, j0:j0 + nj, 64:64 + DH], in_=src_ap)
            if gi == 2:
                if s < 2:
                    zdst = zq if s == 0 else zk
                    nc.sync.dma_start(
                        zdst[ds(shift_rows_sp[s] + 6144 * ttile, 6144),
                             :], stg)
                else:
                    nc.gpsimd.dma_start(
                        zv[ds(shift_rows[2] + 6144 * ttile, 6144), :], stg)



        # ---------- Phase 2: attention per head ----------
        # The exp writes permute the query axis n' -> n'' = 64*(n'%16) + n'//16
        # so that C^T columns land in output-token-friendly order: the final
        # raw reshape sends ctx[h, n', d] to out token n = 64h + n'//16,
        # feature col 48*(n'%16) + d.

        OG_state = {}

        def ogemm_pair(pu, piece, ranges, final, qs=(0, 6)):
            if pu not in OG_state:
                OG_state[pu] = psall.tile([128, DIM], F32, tag="ps",
                                          name=f"pso_{pu}")
            pso = OG_state[pu]
            for c0, cw in ranges:
                for q in range(*qs):
                    nc.tensor.matmul(
                        pso[:, c0:c0 + cw], piece[:, q, :],
                        wo_sb[:, q, c0:c0 + cw],
                        start=(q == 0), stop=(q == 5))
            if final:
                del OG_state[pu]
                outt = work.tile([128, DIM], F32, tag="outt",
                                 name=f"outt_{pu}")
                nc.vector.tensor_add(out=outt, in0=pso, in1=bo_sb)
                nc.sync.dma_start(out[128 * pu:128 * pu + 128, :], outt)

        CTN_state, REC_state, PIECE = {}, {}, {}

        def scatter_q(pu, ctn, hb, ci):
            """Quarter scatter: head hb's ci-half of ctn to concat^T.
            Splitting per ci lets the ci=0 quarters fire inside norm_a, so
            norm_b's reload only waits on the ci=1 quarters."""
            hh = 2 * pu + hb
            plo = 64 * hb
            csrc = ctn[plo:plo + DH, 2 * hb + ci, :] \
                .rearrange("p (j nn) -> p j nn", nn=64)
            cdst = concd[384 * ci:384 * ci + 384, 64 * hh:64 * hh + 64] \
                .rearrange("(j d) nn -> d j nn", d=DH)
            eng = nc.sync if hb == 0 else nc.gpsimd
            eng.dma_start(cdst, csrc)

        def reload_q(pu, piece, hb, ci):
            """Reload the (hb, ci) quarter of the ogemm lhsT piece: ci
            selects q-slabs 3ci..3ci+3 (features 384ci..384ci+384)."""
            hh = 2 * pu + hb
            eng = nc.sync if hb == 0 else nc.gpsimd
            eng.dma_start(
                piece[:, 3 * ci:3 * ci + 3, 64 * hb:64 * hb + 64],
                concd[384 * ci:384 * ci + 384, 64 * hh:64 * hh + 64]
                .rearrange("(q p) n -> p q n", p=128))

        def norm_a(pu):
            """ci=0 half of the pair-pu normalize: the previous pair's
            output GEMM is issued first so the PE covers this pair's C^T
            PSUM drain; then the selS matmul broadcasts the ci=0 raw sums
            (rows 48/112 of sums2) to the 48-row groups of a PSUM tile,
            DVE reciprocal writes it to SBUF bf16 (psB freed immediately;
            bf16 recb makes the scale muls run in 2x DVE mode), and
            ctn[:, ci=0] is scaled.  For the last pair the previous ogemm
            is split: its 512:768 column chain is held back for norm_b so
            the PE has work during the final recip chain."""
            ct = CT_state[pu]
            # Every previous ogemm is split: cols 0:512 here cover this
            # pair's C^T ci=0 PSUM drain, cols 512:768 in norm_b cover the
            # bcast1/recip chain.
            if pu - 1 in PIECE:
                ogemm_pair(pu - 1, PIECE[pu - 1], ((0, 512),),
                           final=False)
            psB = psall.tile([128, 2, 512], F32, tag="ps",
                             name=f"psB_{pu}")
            nc.tensor.matmul(psB[0:112, 0, :], sel2_sb[:, 0:112],
                             sums2[:, 0, :], start=True, stop=True)
            recb = pct.tile([128, 2, 512], BF16, tag="recb", bufs=2,
                            name=f"recb_{pu}")
            with nc.allow_low_precision(reason="bf16 softmax denominators"):
                nc.vector.reciprocal(out=recb[0:DH, 0, :],
                                     in_=psB[0:DH, 0, :])
                nc.vector.reciprocal(out=recb[64:64 + DH, 0, :],
                                     in_=psB[64:64 + DH, 0, :])
            ctn = pct.tile([128, 4, 512], BF16, tag="ctn", bufs=2,
                           name=f"ctn_{pu}")
            nc.vector.tensor_mul(out=ctn[0:DH, 0, :],
                                 in0=ct[0:DH, 0, :], in1=recb[0:DH, 0, :])
            nc.vector.tensor_mul(out=ctn[64:64 + DH, 2, :],
                                 in0=ct[64:64 + DH, 2, :],
                                 in1=recb[64:64 + DH, 0, :])
            piece = work.tile([128, 6, 128], BF16, tag="cpiece", bufs=4,
                              name=f"piece_{pu}")
            for hb in (0, 1):
                scatter_q(pu, ctn, hb, 0)
                reload_q(pu, piece, hb, 0)
            PIECE[pu] = piece
            CTN_state[pu] = ctn
            REC_state[pu] = recb

        def norm_b(pu):
            """ci=1 half + scatter/reload.  Output tokens
            [128*pu, 128*pu+128) depend only on this head pair: scatter
            ctn to concat^T and reload as the ogemm lhsT piece (GEMM
            deferred one pair).  The reload is split per 64-token half so
            it pipelines behind the two scatter writes."""
            ct = CT_state.pop(pu)
            ctn = CTN_state.pop(pu)
            recb = REC_state.pop(pu)
            # PE work first (in-order queue): the previous ogemm's 512:768
            # chain runs while the ci=1 sums drain, THEN the broadcast.
            if pu - 1 in PIECE:
                ogemm_pair(pu - 1, PIECE.pop(pu - 1), ((512, 256),),
                           final=True)
            if pu == 7:
                # The q=0..3 accumulation chains only need the ci=0 piece
                # quarters reloaded back in norm_a -- early PE filler for
                # the final recip/scatter chain.
                ogemm_pair(pu, PIECE[pu], ((0, 512), (512, 256)),
                           final=False, qs=(0, 3))
            psB = psall.tile([128, 2, 512], F32, tag="ps",
                             name=f"psB1_{pu}")
            nc.tensor.matmul(psB[0:112, 0, :], sel2_sb[:, 0:112],
                             sums2[:, 1, :], start=True, stop=True)
            with nc.allow_low_precision(reason="bf16 softmax denominators"):
                nc.vector.reciprocal(out=recb[0:DH, 1, :],
                                     in_=psB[0:DH, 0, :])
                nc.vector.reciprocal(out=recb[64:64 + DH, 1, :],
                                     in_=psB[64:64 + DH, 0, :])
            nc.vector.tensor_mul(out=ctn[0:DH, 1, :],
                                 in0=ct[0:DH, 1, :], in1=recb[0:DH, 1, :])
            nc.vector.tensor_mul(out=ctn[64:64 + DH, 3, :],
                                 in0=ct[64:64 + DH, 3, :],
                                 in1=recb[64:64 + DH, 1, :])
            piece = PIECE[pu]
            for hb in (0, 1):
                # head B's round trip rides the idle gpsimd queue so the
                # two halves overlap instead of serializing on sync
                scatter_q(pu, ctn, hb, 1)
                reload_q(pu, piece, hb, 1)

        S_state, V_state, CT_state, T_state = {}, {}, {}, {}

        def prefetch_T(h):
            """Issue head h's Q^T/K^T transpose loads ahead of first use so
            the ~2us DMA latency hides under the previous head's compute.
            Each tensor is loaded twice (partitions 0-63 / 64-127) from the
            single 64-wide z copy; K^T rides the gpsimd queue."""
            r0 = 32 + SEQ * h
            qt = work.tile([128, SEQ], BF16, tag="qt", bufs=3,
                           name=f"qt_{h}")
            nc.sync.dma_start(qt, zq[r0:r0 + SEQ, :], transpose=True)
            kt = work.tile([128, SEQ], BF16, tag="kt", bufs=3,
                           name=f"kt_{h}")
            nc.sync.dma_start(kt, zk[r0:r0 + SEQ, :], transpose=True)
            T_state[h] = (qt, kt)

        def head_S_start(h):
            if h not in T_state:
                prefetch_T(h)
            qt, kt = T_state.pop(h)
            exps = pexp.tile([128, 4, 2, SEQ], BF16, tag="exps", bufs=4,
                             name=f"exps_{h}")
            return qt, kt, exps

        def emit_S(h, r):
            """S^T + exp for one r (m-tiles r and 4+r, both halves).
            Per-r granularity keeps each (p0=0, p0=64) matmul pair adjacent
            (HW row-group concurrency) while letting the schedule interleave
            other PE work between quarters so the exp drain keeps pace."""
            if h not in S_state:
                S_state[h] = head_S_start(h)
            qt, kt, exps = S_state[h]
            if True:
                for half in range(2):
                    p0 = 64 * half
                    m = half * 4 + r
                    ps = psall.tile([128, 2, 512], F32, tag="ps",
                                    name=f"ps_{h}_{r}_{half}")
                    lhsT = kt[p0:p0 + DH, m * 128:(m + 1) * 128]
                    for ci in range(2):
                        nc.tensor.matmul(
                            ps[:, ci, :], lhsT,
                            qt[p0:p0 + DH, ci * 512:(ci + 1) * 512],
                            start=True, stop=True,
                            tile_position=(p0, 0))
                    # out AP permutes n' -> n'' on write; both APs iterate
                    # (nn outer, j inner) == linear n'.
                    eout = exps[:, r, half, :] \
                        .rearrange("p (j nn) -> p nn j", nn=64)
                    ein = ps.rearrange("p c w -> p (c w)") \
                        .rearrange("p (nn j) -> p nn j", j=16)
                    nc.scalar.activation(out=eout, in_=ein,
                                         func=Exp, scale=SCALE)

        def head_V(h):
            r0 = 32 + SEQ * h
            vt = work.tile([128, 8, DH + 1], BF16, tag="vt", bufs=4,
                           name=f"vt_{h}")
            nc.vector.memset(vt[:, :, DH:DH + 1], 1.0)
            nc.gpsimd.dma_start(
                vt[:, :, 0:DH],
                zv[r0:r0 + SEQ, :].rearrange("(i p) d -> p i d", p=128))
            V_state[h] = vt

        PSC_state = {}

        def emit_C(pu, ci, ih):
            """C^T query-column half ci for pair pu, accumulation i-steps
            ih*4..ih*4+4 (the 8-step chain is split so the schedule can
            interleave S quarters); ones col gives sums on the last step."""
            exps_a = S_state[2 * pu][2]
            exps_b = S_state[2 * pu + 1][2]
            vt_a, vt_b = V_state[2 * pu], V_state[2 * pu + 1]
            if ci == 0 and ih == 0:
                CT_state[pu] = pct.tile([128, 4, 512], BF16, tag="ct",
                                        bufs=2, name=f"ct_{pu}")
            ct = CT_state[pu]
            if ih == 0:
                PSC_state[(pu, ci)] = psall.tile([128, 2, 512], F32,
                                                 tag="ps",
                                                 name=f"psC_{pu}_{ci}")
            psC = PSC_state[(pu, ci)]
            for i in range(4 * ih, 4 * ih + 4):
                rr, hf = i % 4, i // 4
                nc.tensor.matmul(
                    psC[0:DH + 1, 0, :], vt_a[:, i, :],
                    exps_a[:, rr, hf, ci * 512:(ci + 1) * 512],
                    start=(i == 0), stop=(i == 7))
                nc.tensor.matmul(
                    psC[64:64 + DH + 1, 1, :], vt_b[:, i, :],
                    exps_b[:, rr, hf, ci * 512:(ci + 1) * 512],
                    start=(i == 0), stop=(i == 7),
                    tile_position=(0, 64))
            if ih == 0:
                return
            del PSC_state[(pu, ci)]
            # sums first: the normalize broadcast matmul only needs these
            # two, so they must not queue behind the big ct drains on DVE.
            nc.vector.tensor_copy(out=sums2[32:DH + 1, ci, :],
                                  in_=psC[32:DH + 1, 0, :])
            nc.vector.tensor_copy(out=sums2[96:64 + DH + 1, ci, :],
                                  in_=psC[96:64 + DH + 1, 1, :])
            nc.vector.tensor_copy(out=ct[0:DH, ci, :],
                                  in_=psC[0:DH, 0, :])
            nc.vector.tensor_copy(out=ct[64:64 + DH, 2 + ci, :],
                                  in_=psC[64:64 + DH, 1, :])

        # Pipelined schedule: each pair's C^T is delayed one pair and its
        # two query-column halves slot between the next heads' S parts, so
        # the ScalarE exp stream never starves (PSUM can only buffer ~4
        # tiles of exp input).  m-tiles are woven in before the first head
        # that reads their range.
        def blk(u):
            a, b = 2 * u, 2 * u + 1
            it = [("S", a, 0), ("S", a, 1)]
            if u >= 1:
                it += [("C", u - 1, 0, 0)]
            it += [("S", a, 2)]
            if u >= 1:
                it += [("C", u - 1, 0, 1)]
            it += [("S", a, 3), ("V", a), ("S", b, 0)]
            if u >= 1:
                it += [("NA", u - 1)]
            it += [("S", b, 1)]
            if u >= 1:
                it += [("C", u - 1, 1, 0)]
            it += [("S", b, 2)]
            if u >= 1:
                it += [("C", u - 1, 1, 1)]
            it += [("S", b, 3)]
            if u >= 1:
                it += [("NB", u - 1)]
            it += [("V", b)]
            if u < 7:
                it += [("T", a + 2), ("T", b + 2)]
            return it

        def weave(items, extras):
            ex = list(extras)
            out = []
            for x in items:
                out.append(x)
                if ex:
                    out.append(ex.pop(0))
            out.extend(ex)
            return out

        def mg(s, t):
            return [("G", s, t, gi) for gi in range(3)]

        SCHED = mg(0, 0) + mg(1, 0) + [("W",)] + mg(2, 0) + [("L",)]
        SCHED += [("T", 0), ("T", 1)]
        SCHED += mg(0, 1)
        SCHED += weave(blk(0), mg(1, 1))
        SCHED += weave(blk(1), mg(2, 1))
        SCHED += blk(2)
        SCHED += weave(blk(3), mg(0, 2))
        SCHED += weave(blk(4), mg(1, 2))
        SCHED += weave(blk(5), mg(2, 2))
        SCHED += blk(6)
        SCHED += blk(7)
        SCHED += [("C", 7, 0, 0), ("C", 7, 0, 1), ("NA", 7),
                  ("C", 7, 1, 0), ("C", 7, 1, 1), ("NB", 7)]

        for item in SCHED:
            kindi = item[0]
            if kindi == "W":
                # dummy exp on a scratch tile: pulls the ACT table load off
                # the first real exp's critical path (hides in the opener).
                nc.scalar.activation(out=warmt, in_=warmt, func=Exp)
                continue
            if kindi == "L":
                late_loads()
            elif kindi == "G":
                emit_mg(item[1], item[2], item[3])
            elif kindi == "S":
                emit_S(item[1], item[2])
            elif kindi == "V":
                head_V(item[1])
            elif kindi == "C":
                emit_C(item[1], item[2], item[3])
            elif kindi == "T":
                prefetch_T(item[1])
            elif kindi == "NA":
                norm_a(item[1])
            else:
                norm_b(item[1])

        ogemm_pair(7, PIECE.pop(7), ((0, 512), (512, 256)), final=True,
                   qs=(3, 6))


def build_nc(repeat=1):
    nc = bacc.Bacc("TRN2", target_bir_lowering=False, debug=False,
                   num_devices=B)
    xT = nc.dram_tensor("xT", [DIM, TROWS], BF16, kind="ExternalInput").ap()
    wq = nc.dram_tensor("wqkv", [DIM, C3], BF16, kind="ExternalInput").ap()
    wo = nc.dram_tensor("wo", [DIM, DIM], BF16, kind="ExternalInput").ap()
    bo = nc.dram_tensor("bo_b", [128, DIM], F32, kind="ExternalInput").ap()
    sel = nc.dram_tensor("sel", [128, 128],
                         mybir.dt.float32r, kind="ExternalInput").ap()
    out = nc.dram_tensor("out", [SEQ, DIM], F32, kind="ExternalOutput").ap()
    zq = nc.dram_tensor("zq", [ZBUF, 128], BF16).ap()
    zk = nc.dram_tensor("zk", [ZBUF, 128], BF16).ap()
    zv = nc.dram_tensor("zv", [ZBUF, DH], BF16).ap()
    concd = nc.dram_tensor("concd", [DIM, SEQ], BF16).ap()

    with tile.TileContext(nc) as tc:
        for _ in range(repeat):
            _kernel_body(nc, tc, xT, wq, wo, bo, sel,
                         out, zq, zk, zv, concd)
    nc.compile()
    return nc


_NC_CACHE = None


def _get_nc():
    global _NC_CACHE
    if _NC_CACHE is None:
        _NC_CACHE = build_nc()
    return _NC_CACHE


def _t0(s, j):
    # first qkv-GEMM row of core j's range s
    return ((s * 128 + 16 * j) * 64) // 3


def make_in_maps(x, Wqkv, Wo, bo):
    x_flat = np.asarray(x, np.float32).reshape(B * SEQ, DIM)
    wq_bf = np.asarray(Wqkv, np.float32).astype(ml_dtypes.bfloat16)
    wo_bf = np.asarray(Wo, np.float32).astype(ml_dtypes.bfloat16)
    bo_b = np.ascontiguousarray(
        np.broadcast_to(np.asarray(bo, np.float32)[None, :], (128, DIM)))
    # selS broadcasts sums2 row 48 to rows 0-47 and row 112 to rows 64-111
    # of the normalize PSUM tile in a single matmul.
    sel = np.zeros((128, 128), np.float32)
    sel[DH, 0:DH] = 1.0
    sel[64 + DH, 64:64 + DH] = 1.0
    in_maps = []
    for j in range(B):
        rows = np.zeros((TROWS, DIM), np.float32)
        for s in range(3):
            t0 = _t0(s, j)
            t1 = min(t0 + RROWS, B * SEQ)
            rows[s * RROWS: s * RROWS + (t1 - t0)] = x_flat[t0:t1]
        xT = np.ascontiguousarray(rows.T.astype(ml_dtypes.bfloat16))
        in_maps.append({"xT": xT, "wqkv": wq_bf, "wo": wo_bf, "bo_b": bo_b,
                        "sel": sel})
    return in_maps


def kernel(x, Wqkv, Wo, bo):
    global LAST_EXEC_NS, LAST_RESULTS
    nc = _get_nc()
    in_maps = make_in_maps(x, Wqkv, Wo, bo)
    kwargs = {}
    if TRACE:
        kwargs = dict(trace=True,
                      trace_cores=list(range(B)) if TRACE_ALL_CORES else [0])
    res = bass_utils.run_bass_kernel_spmd(
        nc, in_maps, core_ids=list(range(B)), **kwargs)
    LAST_EXEC_NS = res.exec_time_ns
    LAST_RESULTS = res
    out = np.stack([res.results[j]["out"] for j in range(B)], axis=0)
    return np.ascontiguousarray(out.astype(np.float32))



# revision 93
# speedup vs baseline: 2.6165x; 2.6165x over previous
"""Trainium2 Bass kernel for nn_MultiHeadAttention_4999341933079.

Multi-head attention, B=8, N=1024, dim=768, 16 heads, head_dim=48, with the
torch-faithful raw-memory reshapes:
    qkv  = x @ Wqkv                      # [B, N, 2304]
    q,k,v = raw_view(qkv, (3, B, 16, N, 48))
    out  = softmax(q k^T / sqrt(48)) v   -> raw_view -> @ Wo + bo

Sharding: data parallel over the OUTPUT batch.  Core j owns output batch j.
Because the raw reshape scrambles batches, core j's Q/K/V head slabs are three
contiguous flat ranges of the qkv GEMM output.  Each range covers ~342 rows of
the [8192, 2304] GEMM, so core j only computes 3x384 = 1152 GEMM rows (1/8 of
the work, zero collectives, zero redundancy up to padding).

Per-core pipeline (one SPMD program on 8 cores):
  Ph1: qkv GEMM (bf16) -> scatter into DRAM scratch at a dynamic offset
       (core-dependent sub-row shift r_s = (2s + j) % 3, from partition_id):
         zq/zk: [rows, 128] bf16, Q^T/K^T source data duplicated at cols 0-47
                and 64-111 so one DMA-transpose load yields both row-group
                copies for concurrently row-tiled matmuls,
         zv:    [rows, 48] bf16 natural layout.
  Ph2: per head h: prefetched DMA-transpose loads Q^T,K^T [128,1024];
       S^T = K^T.T @ Q^T row-tiled over two 48-row groups; exp via ScalarE
       (scale folded in, no max subtraction -- |S| <= ~6), the write AP
       permuting the query axis into output-token order; C^T = [V|ones].T @
       expS^T col-tiled 2 heads per pass, ones giving softmax denominators.
  Normalize per pair u, split per query-column half ci (norm_a/norm_b):
  a selS matmul broadcasts the raw sums to the 48-row groups, DVE
  reciprocal writes bf16 (so the scale muls run in 2x DVE mode), and ctn
  scatters to concat^T (DRAM) per (head, ci) quarter and reloads as the
  output-GEMM lhsT piece.  The output GEMM  out = concat^T.T @ Wo + bo
  for pair u-1 is split across norm_a (cols 0:512) / norm_b (512:768) as
  PE cover for the drain/recip chains; output tokens 128u..128u+127
  depend only on head pair u.

The schedule interleaves at quarter granularity -- S per r value (2 PSUM
tiles), C per 4-step accumulation half -- because ScalarE's exp
throughput (~1.04us per [128,1024] tile) is the per-pair bottleneck:
every S quarter is followed by non-S PE work so the 4-slot PSUM ring
never parks the PE for long.  qkv m-tile chunk-groups weave into the
first six pairs; Q/K transpose loads prefetch one pair ahead.

Accepts FULL inputs, returns FULL output.  Host work is only
slice/transpose/cast packing (no FLOPs).
"""

import numpy as np
import ml_dtypes

import concourse.mybir as mybir
import concourse.tile as tile
from concourse import bacc
from concourse import bass_utils
from concourse.bass import ds

B = 8
SEQ = 1024
DIM = 768
HEADS = 16
DH = 48
SCALE = DH ** -0.5
C3 = 3 * DIM          # 2304
RROWS = 384           # packed x rows per range (>= 342 actually needed)
TROWS = 3 * RROWS     # 1152
ZBUF = RROWS * 48 + 32  # z-rows per range + shift headroom

BF16 = mybir.dt.bfloat16
F32 = mybir.dt.float32

# qkv GEMM free-dim chunks: 48-aligned (for the scatter copies) and <= 512
# (PSUM bank limit).  Chunks are packed two-per-PSUM-slot ([128, 2, 512] f32,
# chunk i at [:, i, :w] so each stays within one 2KB bank) so an m-tile needs
# 3 pool slots instead of 5 -- the 4-slot psum pool never stalls the PE.
CHUNK_GROUPS = [((0, 480), (480, 480)), ((960, 480), (1440, 480)),
                ((1920, 384),)]

TRACE = False
TRACE_ALL_CORES = True
LAST_EXEC_NS = None
LAST_RESULTS = None


def _kernel_body(nc, tc, xT, wq, wo, bo, sel, out, zq, zk, zv, concd):
    Exp = mybir.ActivationFunctionType.Exp
    F32R = mybir.dt.float32r

    # Core-dependent sub-row shift r_s = (2s + core) % 3 applied as a
    # dynamic DRAM offset on the z-buffer stores.  Each issuing engine
    # needs its own partition_id register (APs bind registers per engine).
    pid = nc.gpsimd.partition_id()
    shift_rows = [32 - 16 * ((2 * s + pid) % 3) for s in range(3)]
    pid_sp = nc.sync.partition_id()
    shift_rows_sp = [32 - 16 * ((2 * s + pid_sp) % 3) for s in range(3)]
    pid_sc = nc.scalar.partition_id()
    shift_rows_sc = [32 - 16 * ((2 * s + pid_sc) % 3) for s in range(3)]

    with tc.tile_pool(name="const", bufs=1) as constp, \
         tc.tile_pool(name="work", bufs=2) as work, \
         tc.tile_pool(name="pexp", bufs=2) as pexp, \
         tc.tile_pool(name="pct", bufs=4) as pct, \
         tc.tile_pool(name="psall", bufs=4, space="PSUM") as psall:
        # Loads are ordered so the phase-1 GEMM streams: xt narrow-first
        # (m-tile s=0 columns of every k land in ~1.6us) on gpsimd, the
        # first chunk-group of wq columns on the sync queue, the remaining
        # wq column groups on the scalar HWDGE queue, and the late-used
        # wo/bo/sel after all wq traffic.
        wo_sb = constp.tile([128, 6, DIM], BF16)
        bo_sb = constp.tile([128, DIM], F32)
        # Persistent normalize state (memset once, rewritten per head pair):
        # sums2 rows 32-48 / 96-112 receive the C^T ones-column copies (sums
        # at rows 48/112, zeros elsewhere); selS rows 48/112 broadcast the
        # sums to the two 48-row groups in one matmul per ci half.
        sums2 = constp.tile([128, 2, 512], F32R)
        sel2_sb = constp.tile([128, 128], F32R)
        xt_sb = constp.tile([128, 6, TROWS], BF16)
        wq_sb = constp.tile([128, 6, C3], BF16)
        xt_r = xT.rearrange("(q p) t -> p q t", p=128)
        wq_r = wq.rearrange("(q p) c -> p q c", p=128)
        GCOLS = [(0, 960), (960, 960), (1920, 384)]
        for k in range(6):
            nc.gpsimd.dma_start(xt_sb[:, k, 0:RROWS], xt_r[:, k, 0:RROWS])
            nc.sync.dma_start(wq_sb[:, k, 0:960], wq_r[:, k, 0:960])
        nc.vector.memset(sums2.bitcast(mybir.dt.uint32), 0)
        warmt = constp.tile([1, 8], F32)
        nc.vector.memset(warmt, 0.0)
        for g0, gw in GCOLS[1:]:
            for k in range(6):
                nc.scalar.dma_start(wq_sb[:, k, g0:g0 + gw],
                                    wq_r[:, k, g0:g0 + gw])
        for k in range(6):
            nc.gpsimd.dma_start(xt_sb[:, k, RROWS:TROWS],
                                xt_r[:, k, RROWS:TROWS])

        def late_loads():
            # Issued mid-schedule (after mg(2,0)) on the gpsimd queue so
            # these transfers never contend with the phase-1 wq/xt streams
            # or block the scalar-engine dup copies.
            nc.gpsimd.dma_start(wo_sb,
                                wo.rearrange("(q p) f -> p q f", p=128))
            nc.gpsimd.dma_start(bo_sb, bo)
            nc.gpsimd.dma_start(sel2_sb, sel)

        # ---------- Phase 1: qkv GEMM + scatter to z buffers ----------
        # Manually-rotated staging buffers; pad cols initialized once.
        # zq/zk data is duplicated at cols 0-47 and 64-111 (the DMA
        # transpose needs a 128-col source, and the two copies feed the
        # concurrently row-tiled S matmuls at partitions 0/64).
        stgs = [constp.tile([128, 48, 128], BF16, name=f"stg{i}")
                for i in range(2)]
        for t in stgs:
            nc.vector.memset(t[:, :, 48:64], 0.0)
            nc.vector.memset(t[:, :, 112:128], 0.0)

        M_state = {}
        _stg_ctr = [0]

        def emit_mg(s, ttile, gi):
            """One chunk-group (~1.9us of PE) of the (s,ttile) m-tile:
            matmuls, drain copies, and on the last group the z scatter."""
            t0 = s * RROWS + ttile * 128
            key = (s, ttile)
            if gi == 0:
                if s < 2:
                    stg = stgs[_stg_ctr[0] % 2]
                    _stg_ctr[0] += 1
                else:
                    stg = work.tile([128, 48, DH], BF16, tag="stgv",
                                    name=f"stgv_{ttile}")
                M_state[key] = stg
            stg = M_state[key]
            grp = CHUNK_GROUPS[gi]
            pg = psall.tile([128, 2, 512], F32, tag="ps",
                            name=f"psc_{s}_{ttile}_{gi}")
            for k in range(6):
                lhsT = xt_sb[:, k, t0:t0 + 128]
                for ci, (c0, cw) in enumerate(grp):
                    nc.tensor.matmul(
                        pg[:, ci, 0:cw], lhsT,
                        wq_sb[:, k, c0:c0 + cw],
                        start=(k == 0), stop=(k == 5))
            for ci, (c0, cw) in enumerate(grp):
                j0, nj = c0 // DH, cw // DH
                src_ap = pg[:, ci, 0:cw].rearrange("p (j d) -> p j d", d=DH)
                nc.vector.tensor_copy(
                    out=stg[:, j0:j0 + nj, 0:DH], in_=src_ap)
                if s < 2:
                    # duplicate row-group copy; ScalarE takes it only for
                    # the pre-blk m-tiles (its queue is exp-only later)
                    if (s, ttile) in ((0, 0), (1, 0), (0, 1)):
                        nc.scalar.copy(
                            out=stg[:, j0:j0 + nj, 64:64 + DH], in_=src_ap)
                    else:
                        nc.vector.tensor_copy(
                            out=stg[:, j0:j0 + nj, 64:64 + DH], in_=src_ap)
            if gi == 2:
                if s < 2:
                    zdst = zq if s == 0 else zk
                    nc.sync.dma_start(
                        zdst[ds(shift_rows_sp[s] + 6144 * ttile, 6144),
                             :], stg)
                else:
                    nc.gpsimd.dma_start(
                        zv[ds(shift_rows[2] + 6144 * ttile, 6144), :], stg)



        # ---------- Phase 2: attention per head ----------
        # The exp writes permute the query axis n' -> n'' = 64*(n'%16) + n'//16
        # so that C^T columns land in output-token-friendly order: the final
        # raw reshape sends ctx[h, n', d] to out token n = 64h + n'//16,
        # feature col 48*(n'%16) + d.

        OG_state = {}

        def ogemm_pair(pu, piece, ranges, final, qs=(0, 6)):
            if pu not in OG_state:
                OG_state[pu] = psall.tile([128, DIM], F32, tag="ps",
                                          name=f"pso_{pu}")
            pso = OG_state[pu]
            for c0, cw in ranges:
                for q in range(*qs):
                    nc.tensor.matmul(
                        pso[:, c0:c0 + cw], piece[:, q, :],
                        wo_sb[:, q, c0:c0 + cw],
                        start=(q == 0), stop=(q == 5))
            if final:
                del OG_state[pu]
                outt = work.tile([128, DIM], F32, tag="outt",
                                 name=f"outt_{pu}")
                nc.vector.tensor_add(out=outt, in0=pso, in1=bo_sb)
                nc.sync.dma_start(out[128 * pu:128 * pu + 128, :], outt)

        CTN_state, REC_state, PIECE = {}, {}, {}

        def scatter_q(pu, ctn, hb, ci):
            """Quarter scatter: head hb's ci-half of ctn to concat^T.
            Splitting per ci lets the ci=0 quarters fire inside norm_a, so
            norm_b's reload only waits on the ci=1 quarters."""
            hh = 2 * pu + hb
            plo = 64 * hb
            csrc = ctn[plo:plo + DH, 2 * hb + ci, :] \
                .rearrange("p (j nn) -> p j nn", nn=64)
            cdst = concd[384 * ci:384 * ci + 384, 64 * hh:64 * hh + 64] \
                .rearrange("(j d) nn -> d j nn", d=DH)
            eng = nc.sync if hb == 0 else nc.gpsimd
            eng.dma_start(cdst, csrc)

        def reload_q(pu, piece, hb, ci):
            """Reload the (hb, ci) quarter of the ogemm lhsT piece: ci
            selects q-slabs 3ci..3ci+3 (features 384ci..384ci+384)."""
            hh = 2 * pu + hb
            eng = nc.sync if hb == 0 else nc.gpsimd
            eng.dma_start(
                piece[:, 3 * ci:3 * ci + 3, 64 * hb:64 * hb + 64],
                concd[384 * ci:384 * ci + 384, 64 * hh:64 * hh + 64]
                .rearrange("(q p) n -> p q n", p=128))

        def norm_a(pu):
            """ci=0 half of the pair-pu normalize: the previous pair's
            output GEMM is issued first so the PE covers this pair's C^T
            PSUM drain; then the selS matmul broadcasts the ci=0 raw sums
            (rows 48/112 of sums2) to the 48-row groups of a PSUM tile,
            DVE reciprocal writes it to SBUF bf16 (psB freed immediately;
            bf16 recb makes the scale muls run in 2x DVE mode), and
            ctn[:, ci=0] is scaled.  For the last pair the previous ogemm
            is split: its 512:768 column chain is held back for norm_b so
            the PE has work during the final recip chain."""
            ct = CT_state[pu]
            # Every previous ogemm is split: cols 0:512 here cover this
            # pair's C^T ci=0 PSUM drain, cols 512:768 in norm_b cover the
            # bcast1/recip chain.
            if pu - 1 in PIECE:
                ogemm_pair(pu - 1, PIECE[pu - 1], ((0, 512),),
                           final=False)
            psB = psall.tile([128, 2, 512], F32, tag="ps",
                             name=f"psB_{pu}")
            nc.tensor.matmul(psB[0:112, 0, :], sel2_sb[:, 0:112],
                             sums2[:, 0, :], start=True, stop=True)
            recb = pct.tile([128, 2, 512], BF16, tag="recb", bufs=2,
                            name=f"recb_{pu}")
            with nc.allow_low_precision(reason="bf16 softmax denominators"):
                nc.vector.reciprocal(out=recb[0:DH, 0, :],
                                     in_=psB[0:DH, 0, :])
                nc.vector.reciprocal(out=recb[64:64 + DH, 0, :],
                                     in_=psB[64:64 + DH, 0, :])
            ctn = pct.tile([128, 4, 512], BF16, tag="ctn", bufs=2,
                           name=f"ctn_{pu}")
            nc.vector.tensor_mul(out=ctn[0:DH, 0, :],
                                 in0=ct[0:DH, 0, :], in1=recb[0:DH, 0, :])
            nc.vector.tensor_mul(out=ctn[64:64 + DH, 2, :],
                                 in0=ct[64:64 + DH, 2, :],
                                 in1=recb[64:64 + DH, 0, :])
            piece = work.tile([128, 6, 128], BF16, tag="cpiece", bufs=4,
                              name=f"piece_{pu}")
            for hb in (0, 1):
                scatter_q(pu, ctn, hb, 0)
                reload_q(pu, piece, hb, 0)
            PIECE[pu] = piece
            CTN_state[pu] = ctn
            REC_state[pu] = recb

        def norm_b(pu):
            """ci=1 half + scatter/reload.  Output tokens
            [128*pu, 128*pu+128) depend only on this head pair: scatter
            ctn to concat^T and reload as the ogemm lhsT piece (GEMM
            deferred one pair).  The reload is split per 64-token half so
            it pipelines behind the two scatter writes."""
            ct = CT_state.pop(pu)
            ctn = CTN_state.pop(pu)
            recb = REC_state.pop(pu)
            # PE work first (in-order queue): the previous ogemm's 512:768
            # chain runs while the ci=1 sums drain, THEN the broadcast.
            if pu - 1 in PIECE:
                ogemm_pair(pu - 1, PIECE.pop(pu - 1), ((512, 256),),
                           final=True)
            if pu == 7:
                # The q=0..3 accumulation chains only need the ci=0 piece
                # quarters reloaded back in norm_a -- early PE filler for
                # the final recip/scatter chain.
                ogemm_pair(pu, PIECE[pu], ((0, 512), (512, 256)),
                           final=False, qs=(0, 3))
            psB = psall.tile([128, 2, 512], F32, tag="ps",
                             name=f"psB1_{pu}")
            nc.tensor.matmul(psB[0:112, 0, :], sel2_sb[:, 0:112],
                             sums2[:, 1, :], start=True, stop=True)
            with nc.allow_low_precision(reason="bf16 softmax denominators"):
                nc.vector.reciprocal(out=recb[0:DH, 1, :],
                                     in_=psB[0:DH, 0, :])
                nc.vector.reciprocal(out=recb[64:64 + DH, 1, :],
                                     in_=psB[64:64 + DH, 0, :])
            nc.vector.tensor_mul(out=ctn[0:DH, 1, :],
                                 in0=ct[0:DH, 1, :], in1=recb[0:DH, 1, :])
            nc.vector.tensor_mul(out=ctn[64:64 + DH, 3, :],
                                 in0=ct[64:64 + DH, 3, :],
                                 in1=recb[64:64 + DH, 1, :])
            piece = PIECE[pu]
            for hb in (0, 1):
                # head B's round trip rides the idle gpsimd queue so the
                # two halves overlap instead of serializing on sync
                scatter_q(pu, ctn, hb, 1)
                reload_q(pu, piece, hb, 1)

        S_state, V_state, CT_state, T_state = {}, {}, {}, {}

        def prefetch_T(h):
            """Issue head h's Q^T/K^T transpose loads ahead of first use so
            the ~2us DMA latency hides under the previous head's compute.
            Each tensor is loaded twice (partitions 0-63 / 64-127) from the
            single 64-wide z copy; K^T rides the gpsimd queue."""
            r0 = 32 + SEQ * h
            qt = work.tile([128, SEQ], BF16, tag="qt", bufs=3,
                           name=f"qt_{h}")
            nc.sync.dma_start(qt, zq[r0:r0 + SEQ, :], transpose=True)
            kt = work.tile([128, SEQ], BF16, tag="kt", bufs=3,
                           name=f"kt_{h}")
            nc.sync.dma_start(kt, zk[r0:r0 + SEQ, :], transpose=True)
            T_state[h] = (qt, kt)

        def head_S_start(h):
            if h not in T_state:
                prefetch_T(h)
            qt, kt = T_state.pop(h)
            exps = pexp.tile([128, 4, 2, SEQ], BF16, tag="exps", bufs=4,
                             name=f"exps_{h}")
            return qt, kt, exps

        def emit_S(h, r):
            """S^T + exp for one r (m-tiles r and 4+r, both halves).
            Per-r granularity keeps each (p0=0, p0=64) matmul pair adjacent
            (HW row-group concurrency) while letting the schedule interleave
            other PE work between quarters so the exp drain keeps pace."""
            if h not in S_state:
                S_state[h] = head_S_start(h)
            qt, kt, exps = S_state[h]
            if True:
                for half in range(2):
                    p0 = 64 * half
                    m = half * 4 + r
                    ps = psall.tile([128, 2, 512], F32, tag="ps",
                                    name=f"ps_{h}_{r}_{half}")
                    lhsT = kt[p0:p0 + DH, m * 128:(m + 1) * 128]
                    for ci in range(2):
                        nc.tensor.matmul(
                            ps[:, ci, :], lhsT,
                            qt[p0:p0 + DH, ci * 512:(ci + 1) * 512],
                            start=True, stop=True,
                            tile_position=(p0, 0))
                    # out AP permutes n' -> n'' on write; both APs iterate
                    # (nn outer, j inner) == linear n'.
                    eout = exps[:, r, half, :] \
                        .rearrange("p (j nn) -> p nn j", nn=64)
                    ein = ps.rearrange("p c w -> p (c w)") \
                        .rearrange("p (nn j) -> p nn j", j=16)
                    nc.scalar.activation(out=eout, in_=ein,
                                         func=Exp, scale=SCALE)

        def head_V(h):
            r0 = 32 + SEQ * h
            vt = work.tile([128, 8, DH + 1], BF16, tag="vt", bufs=4,
                           name=f"vt_{h}")
            nc.vector.memset(vt[:, :, DH:DH + 1], 1.0)
            nc.gpsimd.dma_start(
                vt[:, :, 0:DH],
                zv[r0:r0 + SEQ, :].rearrange("(i p) d -> p i d", p=128))
            V_state[h] = vt

        PSC_state = {}

        def emit_C(pu, ci, ih):
            """C^T query-column half ci for pair pu, accumulation i-steps
            ih*4..ih*4+4 (the 8-step chain is split so the schedule can
            interleave S quarters); ones col gives sums on the last step."""
            exps_a = S_state[2 * pu][2]
            exps_b = S_state[2 * pu + 1][2]
            vt_a, vt_b = V_state[2 * pu], V_state[2 * pu + 1]
            if ci == 0 and ih == 0:
                CT_state[pu] = pct.tile([128, 4, 512], BF16, tag="ct",
                                        bufs=2, name=f"ct_{pu}")
            ct = CT_state[pu]
            if ih == 0:
                PSC_state[(pu, ci)] = psall.tile([128, 2, 512], F32,
                                                 tag="ps",
                                                 name=f"psC_{pu}_{ci}")
            psC = PSC_state[(pu, ci)]
            for i in range(4 * ih, 4 * ih + 4):
                rr, hf = i % 4, i // 4
                nc.tensor.matmul(
                    psC[0:DH + 1, 0, :], vt_a[:, i, :],
                    exps_a[:, rr, hf, ci * 512:(ci + 1) * 512],
                    start=(i == 0), stop=(i == 7))
                nc.tensor.matmul(
                    psC[64:64 + DH + 1, 1, :], vt_b[:, i, :],
                    exps_b[:, rr, hf, ci * 512:(ci + 1) * 512],
                    start=(i == 0), stop=(i == 7),
                    tile_position=(0, 64))
            if ih == 0:
                return
            del PSC_state[(pu, ci)]
            # sums first: the normalize broadcast matmul only needs these
            # two, so they must not queue behind the big ct drains on DVE.
            nc.vector.tensor_copy(out=sums2[32:DH + 1, ci, :],
                                  in_=psC[32:DH + 1, 0, :])
            nc.vector.tensor_copy(out=sums2[96:64 + DH + 1, ci, :],
                                  in_=psC[96:64 + DH + 1, 1, :])
            nc.vector.tensor_copy(out=ct[0:DH, ci, :],
                                  in_=psC[0:DH, 0, :])
            nc.vector.tensor_copy(out=ct[64:64 + DH, 2 + ci, :],
                                  in_=psC[64:64 + DH, 1, :])

        # Pipelined schedule: each pair's C^T is delayed one pair and its
        # two query-column halves slot between the next heads' S parts, so
        # the ScalarE exp stream never starves (PSUM can only buffer ~4
        # tiles of exp input).  m-tiles are woven in before the first head
        # that reads their range.
        def blk(u):
            a, b = 2 * u, 2 * u + 1
            it = [("S", a, 0), ("S", a, 1)]
            if u >= 1:
                it += [("C", u - 1, 0, 0)]
            it += [("S", a, 2)]
            if u >= 1:
                it += [("C", u - 1, 0, 1)]
            it += [("S", a, 3), ("V", a), ("S", b, 0)]
            if u >= 1:
                it += [("NA", u - 1)]
            it += [("S", b, 1)]
            if u >= 1:
                it += [("C", u - 1, 1, 0)]
            it += [("S", b, 2)]
            if u >= 1:
                it += [("C", u - 1, 1, 1)]
            it += [("S", b, 3)]
            if u >= 1:
                it += [("NB", u - 1)]
            it += [("V", b)]
            if u < 7:
                it += [("T", a + 2), ("T", b + 2)]
            return it

        def weave(items, extras):
            ex = list(extras)
            out = []
            for x in items:
                out.append(x)
                if ex:
                    out.append(ex.pop(0))
            out.extend(ex)
            return out

        def mg(s, t):
            return [("G", s, t, gi) for gi in range(3)]

        SCHED = mg(0, 0) + mg(1, 0) + [("W",)] + mg(2, 0) + [("L",)]
        SCHED += [("T", 0), ("T", 1)]
        SCHED += mg(0, 1)
        SCHED += weave(blk(0), mg(1, 1))
        SCHED += weave(blk(1), mg(2, 1))
        SCHED += blk(2)
        SCHED += weave(blk(3), mg(0, 2))
        SCHED += weave(blk(4), mg(1, 2))
        SCHED += weave(blk(5), mg(2, 2))
        SCHED += blk(6)
        SCHED += blk(7)
        SCHED += [("C", 7, 0, 0), ("C", 7, 0, 1), ("NA", 7),
                  ("C", 7, 1, 0), ("C", 7, 1, 1), ("NB", 7)]

        for item in SCHED:
            kindi = item[0]
            if kindi == "W":
                # dummy exp on a scratch tile: pulls the ACT table load off
                # the first real exp's critical path (hides in the opener).
                nc.scalar.activation(out=warmt, in_=warmt, func=Exp)
                continue
            if kindi == "L":
                late_loads()
            elif kindi == "G":
                emit_mg(item[1], item[2], item[3])
            elif kindi == "S":
                emit_S(item[1], item[2])
            elif kindi == "V":
                head_V(item[1])
            elif kindi == "C":
                emit_C(item[1], item[2], item[3])
            elif kindi == "T":
                prefetch_T(item[1])
            elif kindi == "NA":
                norm_a(item[1])
            else:
                norm_b(item[1])

        ogemm_pair(7, PIECE.pop(7), ((0, 512), (512, 256)), final=True,
                   qs=(3, 6))


def build_nc(repeat=1):
    nc = bacc.Bacc("TRN2", target_bir_lowering=False, debug=False,
                   num_devices=B)
    xT = nc.dram_tensor("xT", [DIM, TROWS], BF16, kind="ExternalInput").ap()
    wq = nc.dram_tensor("wqkv", [DIM, C3], BF16, kind="ExternalInput").ap()
    wo = nc.dram_tensor("wo", [DIM, DIM], BF16, kind="ExternalInput").ap()
    bo = nc.dram_tensor("bo_b", [128, DIM], F32, kind="ExternalInput").ap()
    sel = nc.dram_tensor("sel", [128, 128],
                         mybir.dt.float32r, kind="ExternalInput").ap()
    out = nc.dram_tensor("out", [SEQ, DIM], F32, kind="ExternalOutput").ap()
    zq = nc.dram_tensor("zq", [ZBUF, 128], BF16).ap()
    zk = nc.dram_tensor("zk", [ZBUF, 128], BF16).ap()
    zv = nc.dram_tensor("zv", [ZBUF, DH], BF16).ap()
    concd = nc.dram_tensor("concd", [DIM, SEQ], BF16).ap()

    with tile.TileContext(nc) as tc:
        for _ in range(repeat):
            _kernel_body(nc, tc, xT, wq, wo, bo, sel,
                         out, zq, zk, zv, concd)
    nc.compile()
    return nc


_NC_CACHE = None


def _get_nc():
    global _NC_CACHE
    if _NC_CACHE is None:
        _NC_CACHE = build_nc()
    return _NC_CACHE


def _t0(s, j):
    # first qkv-GEMM row of core j's range s
    return ((s * 128 + 16 * j) * 64) // 3


def make_in_maps(x, Wqkv, Wo, bo):
    x_flat = np.asarray(x, np.float32).reshape(B * SEQ, DIM)
    wq_bf = np.asarray(Wqkv, np.float32).astype(ml_dtypes.bfloat16)
    wo_bf = np.asarray(Wo, np.float32).astype(ml_dtypes.bfloat16)
    bo_b = np.ascontiguousarray(
        np.broadcast_to(np.asarray(bo, np.float32)[None, :], (128, DIM)))
    # selS broadcasts sums2 row 48 to rows 0-47 and row 112 to rows 64-111
    # of the normalize PSUM tile in a single matmul.
    sel = np.zeros((128, 128), np.float32)
    sel[DH, 0:DH] = 1.0
    sel[64 + DH, 64:64 + DH] = 1.0
    in_maps = []
    for j in range(B):
        rows = np.zeros((TROWS, DIM), np.float32)
        for s in range(3):
            t0 = _t0(s, j)
            t1 = min(t0 + RROWS, B * SEQ)
            rows[s * RROWS: s * RROWS + (t1 - t0)] = x_flat[t0:t1]
        xT = np.ascontiguousarray(rows.T.astype(ml_dtypes.bfloat16))
        in_maps.append({"xT": xT, "wqkv": wq_bf, "wo": wo_bf, "bo_b": bo_b,
                        "sel": sel})
    return in_maps


def kernel(x, Wqkv, Wo, bo):
    global LAST_EXEC_NS, LAST_RESULTS
    nc = _get_nc()
    in_maps = make_in_maps(x, Wqkv, Wo, bo)
    kwargs = {}
    if TRACE:
        kwargs = dict(trace=True,
                      trace_cores=list(range(B)) if TRACE_ALL_CORES else [0])
    res = bass_utils.run_bass_kernel_spmd(
        nc, in_maps, core_ids=list(range(B)), **kwargs)
    LAST_EXEC_NS = res.exec_time_ns
    LAST_RESULTS = res
    out = np.stack([res.results[j]["out"] for j in range(B)], axis=0)
    return np.ascontiguousarray(out.astype(np.float32))

